# revision 14
# baseline (speedup 1.0000x reference)
"""Trainium2 Bass kernel: per-tensor asymmetric int8 activation quantization
followed by a linear layer (y = quantize(x) @ W.T + bias).

Sharding (8 cores): 4-way over tokens x 2-way over out_features.

v2 design:
  - The per-tensor quant params (inv_scale, zero point) depend only on
    global min/max of x, which the host already holds in full. They are
    computed host-side in exact fp32 (bit-compatible with the reference's
    jnp math) and passed to each core as a tiny [2] input. This removes
    the entire on-device phase 0 (a second 16.7MB x read per core, 73us
    of serialized DVE reduces, and a 26us 8-byte collective).
  - x is host-retiled to [MB, P, KB*P] per core so each 128-token block
    loads as one 16KB-contiguous run per partition (full DMA rate);
    the baseline's 512B descriptors capped x streaming at ~68GB/s.
  - W is host-retiled to [P, KB*dout] (128KB contiguous per partition).
  - bias is folded in during PSUM eviction on the vector engine, not as
    K=1 matmuls (saves ~14us of PE time + 16 ldweights).
  - blocks 0 and 1 are emitted kb-interleaved so the PE consumes weight
    chunks as they stream in during warm-up instead of stalling.

Each core receives:
  xt   [MB, P, KB*P]  fp32  (token-sharded, block-tiled)
  wt   [P, KB*DOUT_C] fp16  (out_feature-sharded, partition-tiled)
  bias [DOUT_C]       fp16
  qp   [2]            fp32  (inv_scale, MAGIC - zp)
and produces y [TOK_C, DOUT_C] fp32.
"""

import sys

import numpy as np

try:  # the grading environment may or may not have concourse on sys.path
    import concourse  # noqa: F401
except ImportError:  # pragma: no cover
    sys.path.insert(0, "/opt/trn_rl_repo")

P = 128
MAGIC = 12582912.0  # 1.5 * 2**23: fp32 add/sub rounds to nearest-even integer
QMIN, QMAX = -128.0, 127.0

# Full-problem shape (hardcoded per contract; kernel() checks them)
B, S, D_IN, D_OUT = 4, 2048, 4096, 4096
R_SHARDS, G_SHARDS = 4, 2  # token shards x out_feature shards
N_CORES = 8


def build_program(d_in, tok, dout, n_cores=N_CORES):
    """Emit the per-core SPMD program. Returns a compiled Bacc object."""
    from contextlib import ExitStack

    import concourse.bacc as bacc
    import concourse.tile as tile
    from concourse import mybir

    f32, f16 = mybir.dt.float32, mybir.dt.float16
    AF = mybir.ActivationFunctionType
    ALU = mybir.AluOpType

    assert d_in % P == 0 and tok % P == 0
    KB, MB = d_in // P, tok // P
    NMM = min(512, dout)
    assert dout % NMM == 0
    NB = dout // NMM

    nc = bacc.Bacc(
        "TRN2",
        target_bir_lowering=False,
        debug=False,
        num_devices=n_cores,
        enable_asserts=False,
    )

    xt = nc.dram_tensor("xt", [MB, P, KB * P], f32, kind="ExternalInput").ap()
    wt = nc.dram_tensor("wt", [P, KB * dout], f16, kind="ExternalInput").ap()
    bias = nc.dram_tensor("bias", [dout], f16, kind="ExternalInput").ap()
    qp = nc.dram_tensor("qp", [2], f32, kind="ExternalInput").ap()
    y = nc.dram_tensor("y", [tok, dout], f32, kind="ExternalOutput").ap()

    with tile.TileContext(nc) as tc, ExitStack() as ctx:
        wpool = ctx.enter_context(tc.tile_pool(name="w", bufs=1))
        xpool = ctx.enter_context(tc.tile_pool(name="x", bufs=2))
        qpool = ctx.enter_context(tc.tile_pool(name="q", bufs=3))
        opool = ctx.enter_context(tc.tile_pool(name="o", bufs=2))
        spool = ctx.enter_context(tc.tile_pool(name="s", bufs=1))
        ppool = ctx.enter_context(tc.tile_pool(name="ps", bufs=2, space="PSUM"))

        # quant params: load + broadcast to all partitions. The broadcast
        # must be emitted on gpsimd BEFORE the W dma_starts: ring
        # back-pressure blocks the issuing engine after ~5 outstanding
        # transfers, and block 0's ACT quant needs bc early.
        qp_row = spool.tile([1, 2], f32)
        nc.sync.dma_start(qp_row[:], qp[None, :])
        bc = spool.tile([P, 2], f32)
        nc.gpsimd.partition_broadcast(bc[:], qp_row[:], channels=P)

        # bias row load (cheap, single instr on the otherwise-idle scalar
        # DMA slot; its partition broadcast can wait until W issue drains)
        bias_bc = wpool.tile([P, dout], f16)
        nc.scalar.dma_start(bias_bc[0:1, :], bias[None, :])

        def load_x(mb):
            x_m = xpool.tile([P, KB * P], f32, tag="xm")
            dma = nc.sync.dma_start(x_m[:], xt[mb])
            return x_m, dma

        # block 0/1 x tiles first: the first matmul is gated on q0, so x0
        # must win the fabric before the W stream starts
        x0, x0_dma = load_x(0)
        x1, _ = load_x(1)

        # resident weights, streamed in kb-order chunks. On the gpsimd
        # queue: the scalar/ACT engine must stay free for per-block quant,
        # and sync carries the x stream.
        w_sb = wpool.tile([P, KB, dout], f16)
        w_view = wt.rearrange("p (kb o) -> p kb o", kb=KB)
        W_CHUNKS = 16
        step = max(1, KB // W_CHUNKS)
        for k0 in range(0, KB, step):
            k1 = min(KB, k0 + step)
            wdma = nc.gpsimd.dma_start(w_sb[:, k0:k1, :], w_view[:, k0:k1, :])
            if k0 == 0:
                tile.add_dep_helper(wdma.ins, x0_dma.ins, reason="x0 first")

        # bias broadcast for the eviction add (needed ~first evict only)
        nc.gpsimd.partition_broadcast(bias_bc[:], bias_bc[0:1, :], channels=P)

        # ---- main loop: quantize + matmul per 128-token block ----
        def quantize(x_m):
            # v = x * inv_scale + MAGIC (ACT); upper bits hold rne(x/scale)
            nc.scalar.activation(x_m[:], x_m[:], AF.Copy, bias=MAGIC, scale=bc[:, 0:1])
            # v - (MAGIC - zp) = rne(x/scale) + zp ; clamp low
            nc.vector.tensor_scalar(
                x_m[:], x_m[:], bc[:, 1:2], QMIN, op0=ALU.subtract, op1=ALU.max
            )
            q_m = qpool.tile([P, KB, P], f16, tag="qm")
            x_m3 = x_m.rearrange("p (a b) -> p a b", b=P)
            nc.vector.tensor_scalar(q_m[:], x_m3, QMAX, None, op0=ALU.min)
            return q_m

        def load_and_quantize(mb):
            x_m, _ = load_x(mb)
            return quantize(x_m)

        def mm_block(psum, q_m, kb):
            lhsT = q_m[:, kb, :]
            for n in range(NB):
                nc.tensor.matmul(
                    psum[:, n * NMM : (n + 1) * NMM],
                    lhsT,
                    w_sb[:, kb, n * NMM : (n + 1) * NMM],
                    start=(kb == 0),
                    stop=(kb == KB - 1),
                )

        def evict(psum, mb, split=1):
            # split>1 halves the evict+writeback latency on the final block
            o_m = opool.tile([P, dout], f32, tag="o_m")
            w_ = dout // split
            for s in range(split):
                sl = slice(s * w_, (s + 1) * w_)
                nc.vector.scalar_tensor_tensor(
                    o_m[:, sl], psum[:, sl], 1.0, bias_bc[:, sl],
                    op0=ALU.mult, op1=ALU.add,
                )
                nc.gpsimd.dma_start(y[mb * P : (mb + 1) * P, sl], o_m[:, sl])

        # Software pipeline. Quantize runs one block ahead of the PE, and
        # each block's evict (which waits on that block's final matmul) is
        # emitted AFTER the next block's quantize ops, so the in-order DVE
        # queue never parks an evict in front of a quantize the PE needs.
        # Warm-up: blocks 0 and 1 kb-interleaved, tracking the W stream.
        q = {0: quantize(x0), 1: quantize(x1)}
        ps0 = ppool.tile([P, dout], f32, tag="psum")
        ps1 = ppool.tile([P, dout], f32, tag="psum")
        for kb in range(KB):
            mm_block(ps0, q[0], kb)
            mm_block(ps1, q[1], kb)
        q[2] = load_and_quantize(2)
        evict(ps0, 0)
        evict(ps1, 1)

        for mb in range(2, MB):
            if mb + 1 < MB:
                q[mb + 1] = load_and_quantize(mb + 1)
            psum = ppool.tile([P, dout], f32, tag="psum")
            for kb in range(KB):
                mm_block(psum, q[mb], kb)
            evict(psum, mb, split=4 if mb == MB - 1 else 1)

    nc.compile()
    _dedupe_ldweights(nc)
    return nc


def _dedupe_ldweights(nc):
    """Remove back-to-back InstLdweights with identical weight access patterns.

    bacc's matmul split emits one Ldweights per Matmult even when consecutive
    matmuls share the stationary operand (our 4 n-slices per k-block). The PE
    keeps the stationary operand loaded between matmuls, so a repeat load with
    the same AP is pure overhead (~108ns each, ~half exposed). Only drop
    loads that carry no semaphore waits/updates.
    """
    from concourse import mybir

    for fn in nc.m.functions:
        for bb in fn.blocks:
            insts = bb.instructions
            keep = []
            last_ldw_key = None
            removed = 0
            for inst in insts:
                tname = type(inst).__name__
                if tname == "InstLdweights":
                    key = inst.concise()
                    if (
                        key == last_ldw_key
                        and not inst.has_wait()
                        and not inst.has_update()
                    ):
                        removed += 1
                        continue
                    last_ldw_key = key
                elif tname == "InstMatmult":
                    pass  # matmuls stream; they don't disturb loaded weights
                elif getattr(inst, "engine", None) == mybir.EngineType.PE and tname not in (
                    "InstEventSemaphore",
                    "InstNop",
                ):
                    # any other PE instruction: be conservative
                    last_ldw_key = None
                keep.append(inst)
            if removed:
                del insts[:]
                for inst in keep:
                    insts.append(inst)


def quant_params(x):
    """Exact fp32 replication of the reference's per-tensor quant math."""
    x = np.asarray(x)
    xmin = x.min().astype(np.float32)
    xmax = x.max().astype(np.float32)
    scale = (xmax - xmin) / np.float32(QMAX - QMIN)
    inv_scale = np.float32(1.0) / scale
    zp = np.clip(
        np.float32(QMIN) - np.round(xmin / scale), np.float32(QMIN), np.float32(QMAX)
    ).astype(np.float32)
    mzp = np.float32(MAGIC) - zp
    return np.array([inv_scale, mzp], dtype=np.float32)


def make_in_maps(x, weight, bias, r_shards=R_SHARDS, g_shards=G_SHARDS):
    """Host-side shard/layout prep. Returns (in_maps, tok_c, dout_c)."""
    x = np.asarray(x, dtype=np.float32)
    weight = np.asarray(weight, dtype=np.float32)
    bias = np.asarray(bias, dtype=np.float32)
    tok_tot = int(np.prod(x.shape[:-1]))
    d_in = x.shape[-1]
    d_out = weight.shape[0]
    tok_c = tok_tot // r_shards
    dout_c = d_out // g_shards
    KB, MB = d_in // P, tok_c // P

    qp = quant_params(x)

    x2 = x.reshape(tok_tot, d_in)
    # per r-shard: [MB, P(d_in sub), KB, P(tok sub)] with x_t[mb,p,kb,t]
    # = x2[r*tok_c + mb*P + t, kb*P + p]; one 16KB-contiguous run per
    # partition per block.
    x_tiles = []
    for r in range(r_shards):
        xr = x2[r * tok_c : (r + 1) * tok_c].reshape(MB, P, KB, P)  # [mb,t,kb,p]
        x_tiles.append(
            np.ascontiguousarray(xr.transpose(0, 3, 2, 1)).reshape(MB, P, KB * P)
        )

    b16 = bias.astype(np.float16)
    w_tiles = []
    for g in range(g_shards):
        wgT = weight[g * dout_c : (g + 1) * dout_c, :].T  # [d_in, dout_c]
        wg = wgT.reshape(KB, P, dout_c).transpose(1, 0, 2)  # [p, kb, o]
        w_tiles.append(np.ascontiguousarray(wg.astype(np.float16)).reshape(P, KB * dout_c))

    in_maps = []
    for c in range(r_shards * g_shards):
        r, g = divmod(c, g_shards)
        in_maps.append(
            {
                "xt": x_tiles[r],
                "wt": w_tiles[g],
                "bias": np.ascontiguousarray(b16[g * dout_c : (g + 1) * dout_c]),
                "qp": qp,
            }
        )
    return in_maps, tok_c, dout_c


def assemble_output(results, out_shape, tok_c, dout_c, g_shards=G_SHARDS):
    d_out = out_shape[-1]
    tok_tot = int(np.prod(out_shape[:-1]))
    Y = np.empty((tok_tot, d_out), np.float32)
    for c, res in enumerate(results):
        r, g = divmod(c, g_shards)
        Y[r * tok_c : (r + 1) * tok_c, g * dout_c : (g + 1) * dout_c] = res["y"]
    return Y.reshape(out_shape)


_PROGRAM_CACHE = {}


def _get_program(d_in, tok_c, dout_c):
    key = (d_in, tok_c, dout_c)
    if key not in _PROGRAM_CACHE:
        _PROGRAM_CACHE[key] = build_program(d_in, tok_c, dout_c, N_CORES)
    return _PROGRAM_CACHE[key]


def kernel(x, weight, bias, trace=False, **_ignored):
    """Full-input entry point: shards across 8 NeuronCores, runs, gathers."""
    from concourse.bass_utils import run_bass_kernel_spmd

    assert x.shape == (B, S, D_IN) and weight.shape == (D_OUT, D_IN)
    in_maps, tok_c, dout_c = make_in_maps(x, weight, bias)
    nc = _get_program(D_IN, tok_c, dout_c)
    out = run_bass_kernel_spmd(nc, in_maps, list(range(N_CORES)), trace=trace)
    res = assemble_output(out.results, (B, S, D_OUT), tok_c, dout_c)
    if trace:
        return res, out
    return res
